# revision 16
# baseline (speedup 1.0000x reference)
"""Bass/Trainium2 kernel for nn_GNNPolicy_MILP (gnn_message_passing).

Strategy (8 NeuronCores, SPMD):
  - Host: cheap scalar graph prep on the nnz/constraint axis (segment sums via
    bincount, ~1.5% of total FLOPs), producing per-node z-inputs inv_s_v, x0,
    s_v. Nodes padded 100000 -> 100352 = 8*12544 and row-sharded per core.
  - Device (per core, fp32): two 128-wide embedding MLPs over 12544 nodes
    (feature-major layout, nodes in the matmul free dim), two conv updates
    with a [128] AllReduce each (global weighted node reduction), 3-layer
    output head. All dense FLOPs (~61 GFLOP total) run on the PE array.

Key algebraic reductions vs the reference (exact, not approximations):
  - emb_rhs is dead code; s_c/scaled_coef/s_v are identical across both convs.
  - mean(x_const) == (1/E) * sum_n s_v[n] * x_var[n]  -- the [50k,128]
    gather/scatter collapses to a weighted reduction over nodes.

Launch-path optimizations (the dominant cost in this axon-tunneled setup is
per-call dispatch, not device compute):
  - jax persistent compilation cache: run_bass_kernel_spmd builds a fresh
    jax.jit closure per call, so without the cache every warm call re-runs
    the BIR->NEFF pipeline (~0.4s walrus + DVE tables). With it, repeat
    calls deserialize the compiled executable keyed on HLO hash.
  - warm kernel() calls reuse ONE cached pjit callable (C++ fastpath, no
    retrace), keep the 1MB weight pack device-resident, and gather the
    sharded output terminal-side so the host does a single fetch.
"""
import hashlib

import numpy as np

import jax

jax.config.update("jax_compilation_cache_dir", "/tmp/bass_jax_cache")
jax.config.update("jax_persistent_cache_min_compile_time_secs", 0.0)
jax.config.update("jax_persistent_cache_min_entry_size_bytes", 0)

import concourse.bass as bass
from concourse import bacc
import concourse.mybir as mybir
import concourse.tile as tile
from concourse.bass_utils import run_bass_kernel_spmd

NUM_NODES = 100000
NUM_EDGES = 50000
DEG = 16
HID = 128
NCORES = 8
NSH = 12544            # padded nodes per core (8*12544 = 100352)
NT = NSH // 128        # 98 rows of the [98,128] z layout
F32 = mybir.dt.float32

_CACHE = {}

_WLIST = [("pc2", 1, 96), ("b96", 96, 1), ("nw", 96, 64), ("nb", 64, 1),
          ("mw1", 64, 256), ("mb1", 128, 2), ("mw2", 128, 512), ("mb2", 128, 2),
          ("mw3", 128, 256), ("mb3", 128, 1), ("linw", 128, 256), ("linb", 128, 2),
          ("actw", 128, 256), ("actb", 128, 2), ("ow1", 128, 128), ("ob1", 128, 1),
          ("ow2", 128, 128), ("ob2", 128, 1), ("ow3", 128, 1), ("ob3", 1, 1),
          ("sig", 1, 2)]
WSPEC = {}
_o = 0
for _n, _r, _c in _WLIST:
    WSPEC[_n] = (_r, _c, _o)
    _o += _c
WCOLS = _o


# --------------------------------------------------------------------- host
def _host_prep(hyperedge_index, coef, rhs):
    row = np.asarray(hyperedge_index[0]).astype(np.int64)
    coef = np.asarray(coef, np.float32)
    rhs = np.asarray(rhs, np.float32).reshape(-1)

    cmat = coef.reshape(NUM_EDGES, DEG)
    s_c = np.abs(cmat).sum(1, dtype=np.float32)
    inv_s_c = np.where(s_c == 0, np.float32(0), np.float32(1) / s_c).astype(np.float32)
    sc = cmat * inv_s_c[:, None]
    rhs1 = rhs * inv_s_c
    rhs2 = rhs1 * inv_s_c
    sig1 = np.float32(rhs1.sum(dtype=np.float64))
    sig2 = np.float32(rhs2.sum(dtype=np.float64))

    s_v = np.bincount(row, weights=sc.ravel(), minlength=NUM_NODES).astype(np.float32)
    x0pre = np.bincount(row, weights=(sc * rhs1[:, None]).ravel(),
                        minlength=NUM_NODES).astype(np.float32)
    with np.errstate(divide="ignore"):
        inv_s_v = np.where(s_v == 0, np.float32(0),
                           np.float32(1) / s_v).astype(np.float32)
    x0 = (inv_s_v * x0pre).astype(np.float32)

    # zall[core] rows: 0 = inv_s_v (raw), 1 = x0 (raw), 2 = s_v
    zflat = np.zeros((3, NCORES * NSH), np.float32)
    zflat[0, :NUM_NODES] = inv_s_v
    zflat[1, :NUM_NODES] = x0
    zflat[2, :NUM_NODES] = s_v
    zall = np.ascontiguousarray(
        zflat.reshape(3, NCORES, NSH).transpose(1, 0, 2))
    return zall, sig1, sig2


# ------------------------------------------------------------------- device
def _build_nc():
    nc = bacc.Bacc(None, num_devices=NCORES)

    zall_d = nc.dram_tensor("zall", [3, NSH], F32, kind="ExternalInput")
    wpk_d = nc.dram_tensor("wpk", [128, WCOLS], F32, kind="ExternalInput")
    out_d = nc.dram_tensor("out", [1, NSH], F32, kind="ExternalOutput")

    AF = mybir.ActivationFunctionType
    ALU = mybir.AluOpType
    RG = [list(range(NCORES))]
    PI2 = float(2 * np.pi)

    with tile.TileContext(nc) as tc:
        with (
            tc.tile_pool(name="persist", bufs=1) as pp,
            tc.tile_pool(name="work", bufs=2) as wp,
            tc.tile_pool(name="psum", bufs=6, space="PSUM") as pq,
            tc.tile_pool(name="dram", bufs=1, space="DRAM") as dp,
        ):
            # ---- two input DMAs total (weights + z rows)
            wpk = pp.tile([128, WCOLS], F32, tag="wpk")
            nc.sync.dma_start(out=wpk[:], in_=wpk_d[:])

            def wsl(name):
                r, c, o = WSPEC[name]
                return wpk[0:r, o:o + c]

            sig = wsl("sig"); pc2 = wsl("pc2"); b96 = wsl("b96")
            nw = wsl("nw"); nb = wsl("nb")
            mw1 = wsl("mw1"); mb1 = wsl("mb1")
            mw2 = wsl("mw2"); mb2 = wsl("mb2")
            mw3 = wsl("mw3"); mb3 = wsl("mb3")
            linw = wsl("linw"); linb = wsl("linb")
            actw = wsl("actw"); actb = wsl("actb")
            ow1 = wsl("ow1"); ob1 = wsl("ob1")
            ow2 = wsl("ow2"); ob2 = wsl("ob2")
            ow3 = wsl("ow3"); ob3 = wsl("ob3")

            E = pp.tile([128, NSH], F32, tag="E")
            XV = pp.tile([128, NSH], F32, tag="XV")
            ones1 = pp.tile([1, 128], F32, tag="ones1")
            nc.vector.memset(ones1[:], 1.0)

            BLKS = [(b * 512, 512) for b in range(24)] + [(24 * 512, 256)]

            def emb_block(zrow, dst, n0, w):
                """dst[:, n0:n0+w] = emb(z) for nodes n0..n0+w, feature-major.

                q = z*pc lands in PSUM; sin periodicity is restored by
                subtracting trunc(q) (q - k for integer k keeps sin(2*pi*q)
                exact), so the Sin LUT only ever sees |arg| < 2*pi."""
                zr = wp.tile([1, 512], F32, tag="zr")
                nc.sync.dma_start(out=zr[:, :w], in_=zall_d[zrow:zrow + 1, n0:n0 + w])
                p_ps = pq.tile([96, 512], F32, tag="ps")
                nc.tensor.matmul(p_ps[:, :w], lhsT=pc2[:], rhs=zr[:, :w],
                                 start=True, stop=True)
                ki = wp.tile([96, 512], mybir.dt.int32, tag="ki")
                nc.vector.tensor_copy(ki[:, :w], p_ps[:, :w])
                kf = wp.tile([96, 512], F32, tag="kf")
                nc.vector.tensor_copy(kf[:, :w], ki[:, :w])
                q = wp.tile([96, 512], F32, tag="q")
                nc.vector.tensor_tensor(out=q[:, :w], in0=p_ps[:, :w],
                                        in1=kf[:, :w], op=ALU.subtract)
                e = wp.tile([96, 512], F32, tag="e")
                nc.scalar.activation(e[:, :w], q[:, :w], AF.Sin, bias=b96[:],
                                     scale=PI2)
                h0p = pq.tile([64, 512], F32, tag="ps")
                nc.tensor.matmul(h0p[:, :w], lhsT=nw[:], rhs=e[:, :w],
                                 start=True, stop=True)
                h0 = wp.tile([64, 512], F32, tag="h0")
                nc.scalar.activation(h0[:, :w], h0p[:, :w], AF.Relu, bias=nb[:])
                h1 = []
                for m in range(2):
                    hp = pq.tile([128, 512], F32, tag="ps")
                    nc.tensor.matmul(hp[:, :w], lhsT=mw1[:, m * 128:(m + 1) * 128],
                                     rhs=h0[:, :w], start=True, stop=True)
                    h = wp.tile([128, 512], F32, tag=f"h1{m}")
                    nc.scalar.activation(h[:, :w], hp[:, :w], AF.Relu,
                                         bias=mb1[:, m:m + 1])
                    h1.append(h)
                h2 = []
                for m in range(2):
                    hp = pq.tile([128, 512], F32, tag="ps")
                    for kc in range(2):
                        nc.tensor.matmul(
                            hp[:, :w],
                            lhsT=mw2[:, kc * 256 + m * 128:kc * 256 + (m + 1) * 128],
                            rhs=h1[kc][:, :w], start=(kc == 0), stop=(kc == 1))
                    h = wp.tile([128, 512], F32, tag=f"h2{m}")
                    nc.scalar.activation(h[:, :w], hp[:, :w], AF.Relu,
                                         bias=mb2[:, m:m + 1])
                    h2.append(h)
                hp = pq.tile([128, 512], F32, tag="ps")
                for kc in range(2):
                    nc.tensor.matmul(hp[:, :w],
                                     lhsT=mw3[:, kc * 128:(kc + 1) * 128],
                                     rhs=h2[kc][:, :w], start=(kc == 0), stop=(kc == 1))
                nc.scalar.activation(dst[:, n0:n0 + w], hp[:, :w],
                                     AF.Identity, bias=mb3[:])

            for n0, w in BLKS:
                emb_block(0, E, n0, w)
            for n0, w in BLKS:
                emb_block(1, XV, n0, w)

            # ---- emb(sig) -> srhs [128, 2]  (tiny N=2 chain)
            p_ps = pq.tile([96, 2], F32, tag="ps")
            nc.tensor.matmul(p_ps[:], lhsT=pc2[:], rhs=sig[:], start=True, stop=True)
            ski = wp.tile([96, 2], mybir.dt.int32, tag="ki")
            nc.vector.tensor_copy(ski[:], p_ps[:])
            skf = wp.tile([96, 2], F32, tag="kf")
            nc.vector.tensor_copy(skf[:], ski[:])
            sq = wp.tile([96, 2], F32, tag="q")
            nc.vector.tensor_tensor(out=sq[:], in0=p_ps[:], in1=skf[:],
                                    op=ALU.subtract)
            se = wp.tile([96, 2], F32, tag="e")
            nc.scalar.activation(se[:], sq[:], AF.Sin, bias=b96[:], scale=PI2)
            sh0p = pq.tile([64, 2], F32, tag="ps")
            nc.tensor.matmul(sh0p[:], lhsT=nw[:], rhs=se[:], start=True, stop=True)
            sh0 = wp.tile([64, 2], F32, tag="h0")
            nc.scalar.activation(sh0[:], sh0p[:], AF.Relu, bias=nb[:])
            sh1 = []
            for m in range(2):
                hp = pq.tile([128, 2], F32, tag="ps")
                nc.tensor.matmul(hp[:], lhsT=mw1[:, m * 128:(m + 1) * 128],
                                 rhs=sh0[:], start=True, stop=True)
                h = wp.tile([128, 2], F32, tag=f"h1{m}")
                nc.scalar.activation(h[:], hp[:], AF.Relu, bias=mb1[:, m:m + 1])
                sh1.append(h)
            sh2 = []
            for m in range(2):
                hp = pq.tile([128, 2], F32, tag="ps")
                for kc in range(2):
                    nc.tensor.matmul(
                        hp[:], lhsT=mw2[:, kc * 256 + m * 128:kc * 256 + (m + 1) * 128],
                        rhs=sh1[kc][:], start=(kc == 0), stop=(kc == 1))
                h = wp.tile([128, 2], F32, tag=f"h2{m}")
                nc.scalar.activation(h[:], hp[:], AF.Relu, bias=mb2[:, m:m + 1])
                sh2.append(h)
            hp = pq.tile([128, 2], F32, tag="ps")
            for kc in range(2):
                nc.tensor.matmul(hp[:], lhsT=mw3[:, kc * 128:(kc + 1) * 128],
                                 rhs=sh2[kc][:], start=(kc == 0), stop=(kc == 1))
            srhs = pp.tile([128, 2], F32, tag="srhs")
            nc.scalar.activation(srhs[:], hp[:], AF.Identity, bias=mb3[:])

            # ---- two convs, each: global w = sum_n s_v[n]*xv[n,:] via AllReduce
            for conv in range(2):
                wpart = pp.tile([128, 1], F32, tag=f"wpart{conv}")
                nc.vector.memset(wpart[:], 0.0)
                for n0, w in BLKS:
                    zr = wp.tile([1, 512], F32, tag="zr")
                    nc.sync.dma_start(out=zr[:, :w], in_=zall_d[2:3, n0:n0 + w])
                    bc = pq.tile([128, 512], F32, tag="ps")
                    nc.tensor.matmul(bc[:, :w], lhsT=ones1[:],
                                     rhs=zr[:, :w], start=True, stop=True)
                    nc.vector.tensor_tensor(out=bc[:, :w], in0=XV[:, n0:n0 + w],
                                            in1=bc[:, :w], op=ALU.mult)
                    red = wp.tile([128, 1], F32, tag="red")
                    nc.vector.tensor_reduce(red[:], bc[:, :w],
                                            axis=mybir.AxisListType.X, op=ALU.add)
                    nc.vector.tensor_add(out=wpart[:], in0=wpart[:], in1=red[:])

                arin = dp.tile([128, 1], F32, tag=f"arin{conv}")
                arout = dp.tile([128, 1], F32, tag=f"arout{conv}")
                nc.sync.dma_start(out=arin[:], in_=wpart[:])
                nc.gpsimd.collective_compute(
                    "AllReduce", ALU.add, replica_groups=RG,
                    ins=[arin.opt()], outs=[arout.opt()])
                war = pp.tile([128, 1], F32, tag=f"war{conv}")
                nc.sync.dma_start(out=war[:], in_=arout[:])

                wd = wp.tile([128, 1], F32, tag="wd")
                nc.vector.tensor_scalar(out=wd[:], in0=war[:],
                                        scalar1=1.0 / NUM_EDGES, scalar2=None,
                                        op0=ALU.mult)
                agg = pq.tile([128, 1], F32, tag="ps")
                nc.tensor.matmul(agg[:], lhsT=linw[:, conv * 128:(conv + 1) * 128],
                                 rhs=wd[:], start=True, stop=True)
                rr = pp.tile([128, 1], F32, tag=f"rr{conv}")
                # rr = srhs[:,conv] - (agg + linb[:,conv])
                nc.vector.tensor_tensor(out=rr[:], in0=srhs[:, conv:conv + 1],
                                        in1=agg[:], op=ALU.subtract)
                nc.vector.tensor_tensor(out=rr[:], in0=rr[:],
                                        in1=linb[:, conv:conv + 1], op=ALU.subtract)
                awrr = pp.tile([128, 128], F32, tag=f"awrr{conv}")
                nc.vector.tensor_scalar(out=awrr[:],
                                        in0=actw[:, conv * 128:(conv + 1) * 128],
                                        scalar1=rr[:, 0:1], scalar2=None,
                                        op0=ALU.mult)
                for n0, w in BLKS:
                    ps = pq.tile([128, 512], F32, tag="ps")
                    nc.tensor.matmul(ps[:, :w], lhsT=awrr[:], rhs=E[:, n0:n0 + w],
                                     start=True, stop=False)
                    nc.tensor.matmul(ps[:, :w],
                                     lhsT=actw[:, conv * 128:(conv + 1) * 128],
                                     rhs=XV[:, n0:n0 + w], start=False, stop=True)
                    nc.scalar.activation(XV[:, n0:n0 + w], ps[:, :w], AF.Relu,
                                         bias=actb[:, conv:conv + 1])

            # ---- head
            for n0, w in BLKS:
                p1 = pq.tile([128, 512], F32, tag="ps")
                nc.tensor.matmul(p1[:, :w], lhsT=ow1[:], rhs=XV[:, n0:n0 + w],
                                 start=True, stop=True)
                g1 = wp.tile([128, 512], F32, tag="h10")
                nc.scalar.activation(g1[:, :w], p1[:, :w], AF.Relu, bias=ob1[:])
                p2 = pq.tile([128, 512], F32, tag="ps")
                nc.tensor.matmul(p2[:, :w], lhsT=ow2[:], rhs=g1[:, :w],
                                 start=True, stop=True)
                g2 = wp.tile([128, 512], F32, tag="h11")
                nc.scalar.activation(g2[:, :w], p2[:, :w], AF.Relu, bias=ob2[:])
                p3 = pq.tile([1, 512], F32, tag="ps")
                nc.tensor.matmul(p3[:, :w], lhsT=ow3[:], rhs=g2[:, :w],
                                 start=True, stop=True)
                ob = wp.tile([1, 512], F32, tag="ob")
                nc.scalar.activation(ob[:, :w], p3[:, :w], AF.Identity, bias=ob3[:])
                nc.sync.dma_start(out=out_d[0:1, n0:n0 + w], in_=ob[:, :w])
    nc.finalize()
    return nc


# ---------------------------------------------------------- cached fast runner
def _make_runner(nc):
    """Build the same pjit callable run_bass_via_pjrt builds, but ONCE, so
    warm calls take the C++ pjit fastpath (no retrace, no recompile) and the
    weight pack stays device-resident. Mirrors bass2jax.run_bass_via_pjrt."""
    from jax.sharding import Mesh, PartitionSpec, NamedSharding
    from jax.experimental.shard_map import shard_map
    from concourse.bass2jax import _bass_exec_p, install_neuronx_cc_hook
    from concourse import bass2jax

    install_neuronx_cc_hook()
    partition_name = nc.partition_id_tensor.name if nc.partition_id_tensor else None
    in_names, out_names, out_avals, zero_outs = [], [], [], []
    for alloc in nc.m.functions[0].allocations:
        if not isinstance(alloc, mybir.MemoryLocationSet):
            continue
        name = alloc.memorylocations[0].name
        if alloc.kind == "ExternalInput":
            if name != partition_name:
                in_names.append(name)
        elif alloc.kind == "ExternalOutput":
            out_names.append(name)
            shape = tuple(alloc.tensor_shape)
            dtype = mybir.dt.np(alloc.dtype)
            out_avals.append(jax.core.ShapedArray(shape, dtype))
            zero_outs.append(np.zeros(shape, dtype))
    n_params = len(in_names)
    n_outs = len(out_avals)
    all_in_names = list(in_names) + list(out_names)
    if partition_name is not None:
        all_in_names.append(partition_name)
    donate = tuple(range(n_params, n_params + n_outs))

    def _body(*args):
        operands = list(args)
        if partition_name is not None:
            operands.append(bass2jax.partition_id_tensor())
        outs = _bass_exec_p.bind(
            *operands, out_avals=tuple(out_avals), in_names=tuple(all_in_names),
            out_names=tuple(out_names), lowering_input_output_aliases=(),
            sim_require_finite=True, sim_require_nnan=True, nc=nc)
        return tuple(outs)

    devices = jax.devices()[:NCORES]
    mesh = Mesh(np.asarray(devices), ("core",))
    in_specs = (PartitionSpec("core"),) * (n_params + n_outs)
    out_specs = (PartitionSpec("core"),) * len(out_names)
    fn = jax.jit(shard_map(_body, mesh=mesh, in_specs=in_specs,
                           out_specs=out_specs, check_rep=False),
                 donate_argnums=donate, keep_unused=True)
    # terminal-side all-gather: host then fetches ONE shard instead of eight
    rep = NamedSharding(mesh, PartitionSpec(None, None))
    gather = jax.jit(lambda a: a, out_shardings=rep)
    wsh = NamedSharding(mesh, PartitionSpec("core"))
    return dict(fn=fn, gather=gather, in_names=in_names, zero_outs=zero_outs,
                wsh=wsh, mesh=mesh)


def _run_fast(runner, zall_cat, wpk):
    """One warm SPMD execution via the cached pjit callable."""
    wkey = hashlib.md5(wpk.tobytes()).hexdigest()
    if runner.get("wkey") != wkey:
        wcat = np.concatenate([wpk[None]] * NCORES, axis=0).reshape(
            NCORES * 128, WCOLS)
        runner["wdev"] = jax.device_put(wcat, runner["wsh"])
        runner["wdev"].block_until_ready()
        runner["wkey"] = wkey
    args = []
    for nm in runner["in_names"]:
        args.append(runner["wdev"] if nm == "wpk" else zall_cat)
    zz = [np.zeros((NCORES * z.shape[0], *z.shape[1:]), z.dtype)
          for z in runner["zero_outs"]]
    outs = runner["fn"](*args, *zz)
    rep = runner["gather"](outs[0])
    return np.asarray(rep)


# -------------------------------------------------------------------- entry
def _pack_weights(inputs, sig1, sig2):
    pc = np.asarray(inputs["pc"], np.float32).reshape(-1)          # [48]
    vals = {}
    vals["pc2"] = np.concatenate([pc, pc]).reshape(1, 96)
    b96 = np.zeros((96, 1), np.float32); b96[:48] = np.float32(np.pi / 2)
    vals["b96"] = b96
    vals["nw"] = np.asarray(inputs["nw"], np.float32)[0]
    vals["nb"] = np.asarray(inputs["nb"], np.float32).reshape(64, 1)
    vals["mw1"] = np.asarray(inputs["mw1"], np.float32)
    vals["mb1"] = np.asarray(inputs["mb1"], np.float32).reshape(2, 128).T.copy()
    mw2 = np.asarray(inputs["mw2"], np.float32)
    vals["mw2"] = np.concatenate([mw2[:128], mw2[128:]], axis=1)
    vals["mb2"] = np.asarray(inputs["mb2"], np.float32).reshape(2, 128).T.copy()
    mw3 = np.asarray(inputs["mw3"], np.float32)
    vals["mw3"] = np.concatenate([mw3[:128], mw3[128:]], axis=1)
    vals["mb3"] = np.asarray(inputs["mb3"], np.float32).reshape(128, 1)
    linw = np.asarray(inputs["lin_c_w"], np.float32)
    vals["linw"] = np.concatenate([linw[0], linw[1]], axis=1)
    vals["linb"] = np.asarray(inputs["lin_c_b"], np.float32).T.copy()
    actw = np.asarray(inputs["act_w"], np.float32)
    vals["actw"] = np.concatenate([actw[0], actw[1]], axis=1)
    vals["actb"] = np.asarray(inputs["act_b"], np.float32).T.copy()
    vals["ow1"] = np.asarray(inputs["ow1"], np.float32)
    vals["ob1"] = np.asarray(inputs["ob1"], np.float32).reshape(128, 1)
    vals["ow2"] = np.asarray(inputs["ow2"], np.float32)
    vals["ob2"] = np.asarray(inputs["ob2"], np.float32).reshape(128, 1)
    vals["ow3"] = np.asarray(inputs["ow3"], np.float32).reshape(128, 1)
    vals["ob3"] = np.asarray(inputs["ob3"], np.float32).reshape(1, 1)
    vals["sig"] = np.array([[sig1, sig2]], np.float32)

    wpack = np.zeros((128, WCOLS), np.float32)
    for name, (r, c, o) in WSPEC.items():
        wpack[0:r, o:o + c] = vals[name]
    return wpack


def kernel(**inputs) -> np.ndarray:
    zall, sig1, sig2 = _host_prep(
        inputs["hyperedge_index"], inputs["coef"], inputs["rhs"])
    wpack = _pack_weights(inputs, sig1, sig2)

    in_maps = [dict(wpk=wpack, zall=np.ascontiguousarray(zall[p]))
               for p in range(NCORES)]
    _CACHE["in_maps"] = in_maps
    if "nc" not in _CACHE:
        _CACHE["nc"] = _build_nc()
        # First call goes through run_bass_kernel_spmd (compiles the NEFF and
        # seeds the persistent executable cache), then pre-warms the cached
        # fast runner so later calls start on the pjit C++ fastpath.
        res = run_bass_kernel_spmd(_CACHE["nc"], in_maps,
                                   core_ids=list(range(NCORES)))
        _CACHE["runner"] = _make_runner(_CACHE["nc"])
        _run_fast(_CACHE["runner"], zall.reshape(NCORES * 3, NSH), wpack)
        outs = [res.results[p]["out"].reshape(-1) for p in range(NCORES)]
        full = np.concatenate(outs)[:NUM_NODES].astype(np.float32)
        return full.reshape(NUM_NODES, 1)

    zall_cat = zall.reshape(NCORES * 3, NSH)
    rep = _run_fast(_CACHE["runner"], zall_cat, wpack)
    full = rep.reshape(-1)[:NUM_NODES].astype(np.float32)
    return full.reshape(NUM_NODES, 1)


# revision 17
# speedup vs baseline: 1.0603x; 1.0603x over previous
"""Bass/Trainium2 kernel for nn_GNNPolicy_MILP (gnn_message_passing).

Strategy (8 NeuronCores, SPMD):
  - Host: cheap scalar graph prep on the nnz/constraint axis (segment sums via
    bincount, ~1.5% of total FLOPs), producing per-node z-inputs inv_s_v, x0,
    s_v. Nodes padded 100000 -> 100352 = 8*12544 and row-sharded per core.
  - Device (per core, fp32): two 128-wide embedding MLPs over 12544 nodes
    (feature-major layout, nodes in the matmul free dim), two conv updates
    with a [128] AllReduce each (global weighted node reduction), 3-layer
    output head. All dense FLOPs (~61 GFLOP total) run on the PE array.

Key algebraic reductions vs the reference (exact, not approximations):
  - emb_rhs is dead code; s_c/scaled_coef/s_v are identical across both convs.
  - mean(x_const) == (1/E) * sum_n s_v[n] * x_var[n]  -- the [50k,128]
    gather/scatter collapses to a weighted reduction over nodes.

Launch-path optimizations (the dominant cost in this axon-tunneled setup is
per-call dispatch, not device compute):
  - jax persistent compilation cache: run_bass_kernel_spmd builds a fresh
    jax.jit closure per call, so without the cache every warm call re-runs
    the BIR->NEFF pipeline (~0.4s walrus + DVE tables). With it, repeat
    calls deserialize the compiled executable keyed on HLO hash.
  - warm kernel() calls reuse ONE cached pjit callable (C++ fastpath, no
    retrace), keep the 1MB weight pack device-resident, and gather the
    sharded output terminal-side so the host does a single fetch.
"""
import hashlib

import numpy as np

import jax

jax.config.update("jax_compilation_cache_dir", "/tmp/bass_jax_cache")
jax.config.update("jax_persistent_cache_min_compile_time_secs", 0.0)
jax.config.update("jax_persistent_cache_min_entry_size_bytes", 0)

import concourse.bass as bass
from concourse import bacc
import concourse.mybir as mybir
import concourse.tile as tile
from concourse.bass_utils import run_bass_kernel_spmd

NUM_NODES = 100000
NUM_EDGES = 50000
DEG = 16
HID = 128
NCORES = 8
NSH = 12544            # padded nodes per core (8*12544 = 100352)
NT = NSH // 128        # 98 rows of the [98,128] z layout
F32 = mybir.dt.float32

_CACHE = {}

_WLIST = [("pc2", 1, 96), ("b96", 96, 1), ("nw", 96, 64), ("nb", 64, 1),
          ("mw1", 64, 256), ("mb1", 128, 2), ("mw2", 128, 512), ("mb2", 128, 2),
          ("mw3", 128, 256), ("mb3", 128, 1), ("linw", 128, 256), ("linb", 128, 2),
          ("actw", 128, 256), ("actb", 128, 2), ("ow1", 128, 128), ("ob1", 128, 1),
          ("ow2", 128, 128), ("ob2", 128, 1), ("ow3", 128, 1), ("ob3", 1, 1),
          ("sig", 1, 2)]
WSPEC = {}
_o = 0
for _n, _r, _c in _WLIST:
    WSPEC[_n] = (_r, _c, _o)
    _o += _c
WCOLS = _o


# --------------------------------------------------------------------- host
def _host_prep(hyperedge_index, coef, rhs):
    row = np.asarray(hyperedge_index[0])  # int32 is fine for bincount
    coef = np.asarray(coef, np.float32)
    rhs = np.asarray(rhs, np.float32).reshape(-1)

    cmat = coef.reshape(NUM_EDGES, DEG)
    s_c = np.abs(cmat).sum(1, dtype=np.float32)
    inv_s_c = np.where(s_c == 0, np.float32(0), np.float32(1) / s_c).astype(np.float32)
    sc = cmat * inv_s_c[:, None]
    rhs1 = rhs * inv_s_c
    rhs2 = rhs1 * inv_s_c
    sig1 = np.float32(rhs1.sum(dtype=np.float64))
    sig2 = np.float32(rhs2.sum(dtype=np.float64))

    s_v = np.bincount(row, weights=sc.ravel(), minlength=NUM_NODES).astype(np.float32)
    x0pre = np.bincount(row, weights=(sc * rhs1[:, None]).ravel(),
                        minlength=NUM_NODES).astype(np.float32)
    with np.errstate(divide="ignore"):
        inv_s_v = np.where(s_v == 0, np.float32(0),
                           np.float32(1) / s_v).astype(np.float32)
    x0 = (inv_s_v * x0pre).astype(np.float32)

    # zall[core] rows: 0 = inv_s_v (raw), 1 = x0 (raw), 2 = s_v
    zflat = np.zeros((3, NCORES * NSH), np.float32)
    zflat[0, :NUM_NODES] = inv_s_v
    zflat[1, :NUM_NODES] = x0
    zflat[2, :NUM_NODES] = s_v
    zall = np.ascontiguousarray(
        zflat.reshape(3, NCORES, NSH).transpose(1, 0, 2))
    return zall, sig1, sig2


# ------------------------------------------------------------------- device
def _build_nc():
    nc = bacc.Bacc(None, num_devices=NCORES)

    zall_d = nc.dram_tensor("zall", [3, NSH], F32, kind="ExternalInput")
    wpk_d = nc.dram_tensor("wpk", [128, WCOLS], F32, kind="ExternalInput")
    out_d = nc.dram_tensor("out", [1, NSH], F32, kind="ExternalOutput")

    AF = mybir.ActivationFunctionType
    ALU = mybir.AluOpType
    RG = [list(range(NCORES))]
    PI2 = float(2 * np.pi)

    with tile.TileContext(nc) as tc:
        with (
            tc.tile_pool(name="persist", bufs=1) as pp,
            tc.tile_pool(name="work", bufs=3) as wp,
            tc.tile_pool(name="psum", bufs=8, space="PSUM") as pq,
            tc.tile_pool(name="dram", bufs=1, space="DRAM") as dp,
        ):
            # ---- two input DMAs total (weights + z rows)
            wpk = pp.tile([128, WCOLS], F32, tag="wpk")
            nc.sync.dma_start(out=wpk[:], in_=wpk_d[:])

            def wsl(name):
                r, c, o = WSPEC[name]
                return wpk[0:r, o:o + c]

            sig = wsl("sig"); pc2 = wsl("pc2"); b96 = wsl("b96")
            nw = wsl("nw"); nb = wsl("nb")
            mw1 = wsl("mw1"); mb1 = wsl("mb1")
            mw2 = wsl("mw2"); mb2 = wsl("mb2")
            mw3 = wsl("mw3"); mb3 = wsl("mb3")
            linw = wsl("linw"); linb = wsl("linb")
            actw = wsl("actw"); actb = wsl("actb")
            ow1 = wsl("ow1"); ob1 = wsl("ob1")
            ow2 = wsl("ow2"); ob2 = wsl("ob2")
            ow3 = wsl("ow3"); ob3 = wsl("ob3")

            E = pp.tile([128, NSH], F32, tag="E")
            XV = pp.tile([128, NSH], F32, tag="XV")
            ones1 = pp.tile([1, 128], F32, tag="ones1")
            nc.vector.memset(ones1[:], 1.0)

            BLKS = [(b * 512, 512) for b in range(24)] + [(24 * 512, 256)]

            def emb_block(zrow, dst, n0, w):
                """dst[:, n0:n0+w] = emb(z) for nodes n0..n0+w, feature-major.

                q = z*pc lands in PSUM; sin periodicity is restored by
                subtracting trunc(q) (q - k for integer k keeps sin(2*pi*q)
                exact), so the Sin LUT only ever sees |arg| < 2*pi."""
                zr = wp.tile([1, 512], F32, tag="zr")
                nc.sync.dma_start(out=zr[:, :w], in_=zall_d[zrow:zrow + 1, n0:n0 + w])
                p_ps = pq.tile([96, 512], F32, tag="ps")
                nc.tensor.matmul(p_ps[:, :w], lhsT=pc2[:], rhs=zr[:, :w],
                                 start=True, stop=True)
                ki = wp.tile([96, 512], mybir.dt.int32, tag="ki")
                nc.vector.tensor_copy(ki[:, :w], p_ps[:, :w])
                kf = wp.tile([96, 512], F32, tag="kf")
                nc.vector.tensor_copy(kf[:, :w], ki[:, :w])
                q = wp.tile([96, 512], F32, tag="q")
                nc.vector.tensor_tensor(out=q[:, :w], in0=p_ps[:, :w],
                                        in1=kf[:, :w], op=ALU.subtract)
                e = wp.tile([96, 512], F32, tag="e")
                nc.scalar.activation(e[:, :w], q[:, :w], AF.Sin, bias=b96[:],
                                     scale=PI2)
                h0p = pq.tile([64, 512], F32, tag="ps")
                nc.tensor.matmul(h0p[:, :w], lhsT=nw[:], rhs=e[:, :w],
                                 start=True, stop=True)
                h0 = wp.tile([64, 512], F32, tag="h0")
                nc.scalar.activation(h0[:, :w], h0p[:, :w], AF.Relu, bias=nb[:])
                h1 = []
                for m in range(2):
                    hp = pq.tile([128, 512], F32, tag="ps")
                    nc.tensor.matmul(hp[:, :w], lhsT=mw1[:, m * 128:(m + 1) * 128],
                                     rhs=h0[:, :w], start=True, stop=True)
                    h = wp.tile([128, 512], F32, tag=f"h1{m}")
                    nc.scalar.activation(h[:, :w], hp[:, :w], AF.Relu,
                                         bias=mb1[:, m:m + 1])
                    h1.append(h)
                h2 = []
                for m in range(2):
                    hp = pq.tile([128, 512], F32, tag="ps")
                    for kc in range(2):
                        nc.tensor.matmul(
                            hp[:, :w],
                            lhsT=mw2[:, kc * 256 + m * 128:kc * 256 + (m + 1) * 128],
                            rhs=h1[kc][:, :w], start=(kc == 0), stop=(kc == 1))
                    h = wp.tile([128, 512], F32, tag=f"h2{m}")
                    nc.scalar.activation(h[:, :w], hp[:, :w], AF.Relu,
                                         bias=mb2[:, m:m + 1])
                    h2.append(h)
                hp = pq.tile([128, 512], F32, tag="ps")
                for kc in range(2):
                    nc.tensor.matmul(hp[:, :w],
                                     lhsT=mw3[:, kc * 128:(kc + 1) * 128],
                                     rhs=h2[kc][:, :w], start=(kc == 0), stop=(kc == 1))
                nc.scalar.activation(dst[:, n0:n0 + w], hp[:, :w],
                                     AF.Identity, bias=mb3[:])

            for n0, w in BLKS:
                emb_block(0, E, n0, w)
            for n0, w in BLKS:
                emb_block(1, XV, n0, w)

            # ---- emb(sig) -> srhs [128, 2]  (tiny N=2 chain)
            p_ps = pq.tile([96, 2], F32, tag="ps")
            nc.tensor.matmul(p_ps[:], lhsT=pc2[:], rhs=sig[:], start=True, stop=True)
            ski = wp.tile([96, 2], mybir.dt.int32, tag="ki")
            nc.vector.tensor_copy(ski[:], p_ps[:])
            skf = wp.tile([96, 2], F32, tag="kf")
            nc.vector.tensor_copy(skf[:], ski[:])
            sq = wp.tile([96, 2], F32, tag="q")
            nc.vector.tensor_tensor(out=sq[:], in0=p_ps[:], in1=skf[:],
                                    op=ALU.subtract)
            se = wp.tile([96, 2], F32, tag="e")
            nc.scalar.activation(se[:], sq[:], AF.Sin, bias=b96[:], scale=PI2)
            sh0p = pq.tile([64, 2], F32, tag="ps")
            nc.tensor.matmul(sh0p[:], lhsT=nw[:], rhs=se[:], start=True, stop=True)
            sh0 = wp.tile([64, 2], F32, tag="h0")
            nc.scalar.activation(sh0[:], sh0p[:], AF.Relu, bias=nb[:])
            sh1 = []
            for m in range(2):
                hp = pq.tile([128, 2], F32, tag="ps")
                nc.tensor.matmul(hp[:], lhsT=mw1[:, m * 128:(m + 1) * 128],
                                 rhs=sh0[:], start=True, stop=True)
                h = wp.tile([128, 2], F32, tag=f"h1{m}")
                nc.scalar.activation(h[:], hp[:], AF.Relu, bias=mb1[:, m:m + 1])
                sh1.append(h)
            sh2 = []
            for m in range(2):
                hp = pq.tile([128, 2], F32, tag="ps")
                for kc in range(2):
                    nc.tensor.matmul(
                        hp[:], lhsT=mw2[:, kc * 256 + m * 128:kc * 256 + (m + 1) * 128],
                        rhs=sh1[kc][:], start=(kc == 0), stop=(kc == 1))
                h = wp.tile([128, 2], F32, tag=f"h2{m}")
                nc.scalar.activation(h[:], hp[:], AF.Relu, bias=mb2[:, m:m + 1])
                sh2.append(h)
            hp = pq.tile([128, 2], F32, tag="ps")
            for kc in range(2):
                nc.tensor.matmul(hp[:], lhsT=mw3[:, kc * 128:(kc + 1) * 128],
                                 rhs=sh2[kc][:], start=(kc == 0), stop=(kc == 1))
            srhs = pp.tile([128, 2], F32, tag="srhs")
            nc.scalar.activation(srhs[:], hp[:], AF.Identity, bias=mb3[:])

            # ---- two convs, each: global w = sum_n s_v[n]*xv[n,:] via AllReduce
            for conv in range(2):
                wpart = pp.tile([128, 1], F32, tag=f"wpart{conv}")
                nc.vector.memset(wpart[:], 0.0)
                for n0, w in BLKS:
                    zr = wp.tile([1, 512], F32, tag="zr")
                    nc.sync.dma_start(out=zr[:, :w], in_=zall_d[2:3, n0:n0 + w])
                    bc = pq.tile([128, 512], F32, tag="ps")
                    nc.tensor.matmul(bc[:, :w], lhsT=ones1[:],
                                     rhs=zr[:, :w], start=True, stop=True)
                    nc.vector.tensor_tensor(out=bc[:, :w], in0=XV[:, n0:n0 + w],
                                            in1=bc[:, :w], op=ALU.mult)
                    red = wp.tile([128, 1], F32, tag="red")
                    nc.vector.tensor_reduce(red[:], bc[:, :w],
                                            axis=mybir.AxisListType.X, op=ALU.add)
                    nc.vector.tensor_add(out=wpart[:], in0=wpart[:], in1=red[:])

                arin = dp.tile([128, 1], F32, tag=f"arin{conv}")
                arout = dp.tile([128, 1], F32, tag=f"arout{conv}")
                nc.sync.dma_start(out=arin[:], in_=wpart[:])
                nc.gpsimd.collective_compute(
                    "AllReduce", ALU.add, replica_groups=RG,
                    ins=[arin.opt()], outs=[arout.opt()])
                war = pp.tile([128, 1], F32, tag=f"war{conv}")
                nc.sync.dma_start(out=war[:], in_=arout[:])

                wd = wp.tile([128, 1], F32, tag="wd")
                nc.vector.tensor_scalar(out=wd[:], in0=war[:],
                                        scalar1=1.0 / NUM_EDGES, scalar2=None,
                                        op0=ALU.mult)
                agg = pq.tile([128, 1], F32, tag="ps")
                nc.tensor.matmul(agg[:], lhsT=linw[:, conv * 128:(conv + 1) * 128],
                                 rhs=wd[:], start=True, stop=True)
                rr = pp.tile([128, 1], F32, tag=f"rr{conv}")
                # rr = srhs[:,conv] - (agg + linb[:,conv])
                nc.vector.tensor_tensor(out=rr[:], in0=srhs[:, conv:conv + 1],
                                        in1=agg[:], op=ALU.subtract)
                nc.vector.tensor_tensor(out=rr[:], in0=rr[:],
                                        in1=linb[:, conv:conv + 1], op=ALU.subtract)
                awrr = pp.tile([128, 128], F32, tag=f"awrr{conv}")
                nc.vector.tensor_scalar(out=awrr[:],
                                        in0=actw[:, conv * 128:(conv + 1) * 128],
                                        scalar1=rr[:, 0:1], scalar2=None,
                                        op0=ALU.mult)
                for n0, w in BLKS:
                    ps = pq.tile([128, 512], F32, tag="ps")
                    nc.tensor.matmul(ps[:, :w], lhsT=awrr[:], rhs=E[:, n0:n0 + w],
                                     start=True, stop=False)
                    nc.tensor.matmul(ps[:, :w],
                                     lhsT=actw[:, conv * 128:(conv + 1) * 128],
                                     rhs=XV[:, n0:n0 + w], start=False, stop=True)
                    nc.scalar.activation(XV[:, n0:n0 + w], ps[:, :w], AF.Relu,
                                         bias=actb[:, conv:conv + 1])

            # ---- head
            for n0, w in BLKS:
                p1 = pq.tile([128, 512], F32, tag="ps")
                nc.tensor.matmul(p1[:, :w], lhsT=ow1[:], rhs=XV[:, n0:n0 + w],
                                 start=True, stop=True)
                g1 = wp.tile([128, 512], F32, tag="h10")
                nc.scalar.activation(g1[:, :w], p1[:, :w], AF.Relu, bias=ob1[:])
                p2 = pq.tile([128, 512], F32, tag="ps")
                nc.tensor.matmul(p2[:, :w], lhsT=ow2[:], rhs=g1[:, :w],
                                 start=True, stop=True)
                g2 = wp.tile([128, 512], F32, tag="h11")
                nc.scalar.activation(g2[:, :w], p2[:, :w], AF.Relu, bias=ob2[:])
                p3 = pq.tile([1, 512], F32, tag="ps")
                nc.tensor.matmul(p3[:, :w], lhsT=ow3[:], rhs=g2[:, :w],
                                 start=True, stop=True)
                ob = wp.tile([1, 512], F32, tag="ob")
                nc.scalar.activation(ob[:, :w], p3[:, :w], AF.Identity, bias=ob3[:])
                nc.sync.dma_start(out=out_d[0:1, n0:n0 + w], in_=ob[:, :w])
    nc.finalize()
    return nc


# ---------------------------------------------------------- cached fast runner
def _make_runner(nc):
    """Build the same pjit callable run_bass_via_pjrt builds, but ONCE, so
    warm calls take the C++ pjit fastpath (no retrace, no recompile) and the
    weight pack stays device-resident. Mirrors bass2jax.run_bass_via_pjrt."""
    from jax.sharding import Mesh, PartitionSpec, NamedSharding
    from jax.experimental.shard_map import shard_map
    from concourse.bass2jax import _bass_exec_p, install_neuronx_cc_hook
    from concourse import bass2jax

    install_neuronx_cc_hook()
    partition_name = nc.partition_id_tensor.name if nc.partition_id_tensor else None
    in_names, out_names, out_avals, zero_outs = [], [], [], []
    for alloc in nc.m.functions[0].allocations:
        if not isinstance(alloc, mybir.MemoryLocationSet):
            continue
        name = alloc.memorylocations[0].name
        if alloc.kind == "ExternalInput":
            if name != partition_name:
                in_names.append(name)
        elif alloc.kind == "ExternalOutput":
            out_names.append(name)
            shape = tuple(alloc.tensor_shape)
            dtype = mybir.dt.np(alloc.dtype)
            out_avals.append(jax.core.ShapedArray(shape, dtype))
            zero_outs.append(np.zeros(shape, dtype))
    n_params = len(in_names)
    n_outs = len(out_avals)
    all_in_names = list(in_names) + list(out_names)
    if partition_name is not None:
        all_in_names.append(partition_name)
    donate = tuple(range(n_params, n_params + n_outs))

    def _body(*args):
        operands = list(args)
        if partition_name is not None:
            operands.append(bass2jax.partition_id_tensor())
        outs = _bass_exec_p.bind(
            *operands, out_avals=tuple(out_avals), in_names=tuple(all_in_names),
            out_names=tuple(out_names), lowering_input_output_aliases=(),
            sim_require_finite=True, sim_require_nnan=True, nc=nc)
        return tuple(outs)

    devices = jax.devices()[:NCORES]
    mesh = Mesh(np.asarray(devices), ("core",))
    in_specs = (PartitionSpec("core"),) * (n_params + n_outs)
    out_specs = (PartitionSpec("core"),) * len(out_names)
    fn = jax.jit(shard_map(_body, mesh=mesh, in_specs=in_specs,
                           out_specs=out_specs, check_rep=False),
                 donate_argnums=donate, keep_unused=True)
    # terminal-side all-gather: host then fetches ONE shard instead of eight
    rep = NamedSharding(mesh, PartitionSpec(None, None))
    gather = jax.jit(lambda a: a, out_shardings=rep)
    wsh = NamedSharding(mesh, PartitionSpec("core"))
    return dict(fn=fn, gather=gather, in_names=in_names, zero_outs=zero_outs,
                wsh=wsh, mesh=mesh)


def _run_fast(runner, zall_cat, wpk):
    """One warm SPMD execution via the cached pjit callable."""
    wkey = hashlib.md5(wpk.tobytes()).hexdigest()
    if runner.get("wkey") != wkey:
        wcat = np.concatenate([wpk[None]] * NCORES, axis=0).reshape(
            NCORES * 128, WCOLS)
        runner["wdev"] = jax.device_put(wcat, runner["wsh"])
        runner["wdev"].block_until_ready()
        runner["wkey"] = wkey
    args = []
    for nm in runner["in_names"]:
        args.append(runner["wdev"] if nm == "wpk" else zall_cat)
    zz = [np.zeros((NCORES * z.shape[0], *z.shape[1:]), z.dtype)
          for z in runner["zero_outs"]]
    outs = runner["fn"](*args, *zz)
    rep = runner["gather"](outs[0])
    return np.asarray(rep)


# -------------------------------------------------------------------- entry
def _pack_weights(inputs, sig1, sig2):
    pc = np.asarray(inputs["pc"], np.float32).reshape(-1)          # [48]
    vals = {}
    vals["pc2"] = np.concatenate([pc, pc]).reshape(1, 96)
    b96 = np.zeros((96, 1), np.float32); b96[:48] = np.float32(np.pi / 2)
    vals["b96"] = b96
    vals["nw"] = np.asarray(inputs["nw"], np.float32)[0]
    vals["nb"] = np.asarray(inputs["nb"], np.float32).reshape(64, 1)
    vals["mw1"] = np.asarray(inputs["mw1"], np.float32)
    vals["mb1"] = np.asarray(inputs["mb1"], np.float32).reshape(2, 128).T.copy()
    mw2 = np.asarray(inputs["mw2"], np.float32)
    vals["mw2"] = np.concatenate([mw2[:128], mw2[128:]], axis=1)
    vals["mb2"] = np.asarray(inputs["mb2"], np.float32).reshape(2, 128).T.copy()
    mw3 = np.asarray(inputs["mw3"], np.float32)
    vals["mw3"] = np.concatenate([mw3[:128], mw3[128:]], axis=1)
    vals["mb3"] = np.asarray(inputs["mb3"], np.float32).reshape(128, 1)
    linw = np.asarray(inputs["lin_c_w"], np.float32)
    vals["linw"] = np.concatenate([linw[0], linw[1]], axis=1)
    vals["linb"] = np.asarray(inputs["lin_c_b"], np.float32).T.copy()
    actw = np.asarray(inputs["act_w"], np.float32)
    vals["actw"] = np.concatenate([actw[0], actw[1]], axis=1)
    vals["actb"] = np.asarray(inputs["act_b"], np.float32).T.copy()
    vals["ow1"] = np.asarray(inputs["ow1"], np.float32)
    vals["ob1"] = np.asarray(inputs["ob1"], np.float32).reshape(128, 1)
    vals["ow2"] = np.asarray(inputs["ow2"], np.float32)
    vals["ob2"] = np.asarray(inputs["ob2"], np.float32).reshape(128, 1)
    vals["ow3"] = np.asarray(inputs["ow3"], np.float32).reshape(128, 1)
    vals["ob3"] = np.asarray(inputs["ob3"], np.float32).reshape(1, 1)
    vals["sig"] = np.array([[sig1, sig2]], np.float32)

    wpack = np.zeros((128, WCOLS), np.float32)
    for name, (r, c, o) in WSPEC.items():
        wpack[0:r, o:o + c] = vals[name]
    return wpack


def kernel(**inputs) -> np.ndarray:
    zall, sig1, sig2 = _host_prep(
        inputs["hyperedge_index"], inputs["coef"], inputs["rhs"])
    wpack = _pack_weights(inputs, sig1, sig2)

    in_maps = [dict(wpk=wpack, zall=np.ascontiguousarray(zall[p]))
               for p in range(NCORES)]
    _CACHE["in_maps"] = in_maps
    if "nc" not in _CACHE:
        _CACHE["nc"] = _build_nc()
        # First call goes through run_bass_kernel_spmd (compiles the NEFF and
        # seeds the persistent executable cache), then pre-warms the cached
        # fast runner so later calls start on the pjit C++ fastpath.
        res = run_bass_kernel_spmd(_CACHE["nc"], in_maps,
                                   core_ids=list(range(NCORES)))
        _CACHE["runner"] = _make_runner(_CACHE["nc"])
        _run_fast(_CACHE["runner"], zall.reshape(NCORES * 3, NSH), wpack)
        outs = [res.results[p]["out"].reshape(-1) for p in range(NCORES)]
        full = np.concatenate(outs)[:NUM_NODES].astype(np.float32)
        return full.reshape(NUM_NODES, 1)

    zall_cat = zall.reshape(NCORES * 3, NSH)
    rep = _run_fast(_CACHE["runner"], zall_cat, wpack)
    full = rep.reshape(-1)[:NUM_NODES].astype(np.float32)
    return full.reshape(NUM_NODES, 1)
